# revision 3
# baseline (speedup 1.0000x reference)
"""CRF-RNN layer (nn_CrfRnnLayer) as a multi-core Trainium2 Bass kernel.

Strategy (sharding_hint): shard the N=H*W pixel dimension of the Gaussian
kernel rows across the 8 cores.  Each core owns a strip of S=N/8=800 output
pixels.  Phase 0 materializes that core's strips of the two N x N Gaussian
kernels (spatial + bilateral) in bf16 into device DRAM, along with the
normalization row sums.  Phase 1 runs the 5 mean-field iterations: local
softmax over classes, a tiny AllGather of the (800, 21) softmax strips,
then DMA-streams the K strips through accumulating matmuls.

Math trick: K[j, i] = exp(-0.5*||f_j - f_i||^2) = exp(g_j . h_i) with
augmented features g = [f, 1, -0.5||f||^2], h = [f, -0.5||f||^2, 1], so the
kernel tiles come from a single small-contraction matmul followed by Exp on
the scalar engine.  The 21x21 chains are fused on the host:
pairwise = (CM@SK) @ spatial_out + (CM@BK) @ bilateral_out.
"""

import json

import numpy as np

from concourse import bacc, bass, mybir, tile
from concourse.bass_utils import run_bass_kernel_spmd


def _split_bir_multiwaits(bir_json: bytes) -> bytes:
    """Split >1-sync-wait instructions into single-wait chains.

    The staged walrus build allows only one embedded sync-wait per
    instruction; prepend pure-wait EventSemaphores (same engine, same
    block) for all but the last wait.  Tile completion semaphores only
    count up within the kernel epoch, so waiting sequentially is
    equivalent to the simultaneous multi-wait.
    """
    d = json.loads(bir_json)
    for fn in d.get("functions", []):
        for blk in fn.get("blocks", []):
            out = []
            for inst in blk.get("instructions", []):
                si = inst.get("sync_info") or {}
                waits = si.get("on_wait") or []
                if len(waits) > 1:
                    for j, w in enumerate(waits[:-1]):
                        out.append({
                            "debug": inst.get("debug", 0),
                            "engine": inst["engine"],
                            "ins": [],
                            "name": f"{inst['name']}-sw{j}",
                            "opcode": "EventSemaphore",
                            "outs": [],
                            "sync_info": {"on_update": [], "on_wait": [w]},
                        })
                    si["on_wait"] = [waits[-1]]
                out.append(inst)
            blk["instructions"] = out
    return json.dumps(d).encode()


def _install_birpatch():
    if _CACHE.get("birpatch"):
        return
    from concourse import bass2jax
    orig = bass2jax.compile_bir_kernel

    def patched(bir_json, tmpdir, neff_name="file.neff"):
        return orig(_split_bir_multiwaits(bir_json), tmpdir, neff_name)

    bass2jax.compile_bir_kernel = patched
    _CACHE["birpatch"] = True

H = W = 80
C = 21
N = H * W            # 6400
M = 8                # cores
S = N // M           # 800 pixels per core strip
NIT = 5
THETA_ALPHA, THETA_BETA, THETA_GAMMA = 160.0, 3.0, 3.0
DB = 7               # bilateral augmented feature dim (5 + 2)
DS = 4               # spatial augmented feature dim  (2 + 2)
NT = N // 128        # 50 contraction tiles of 128 pixels
CH = 5               # j-tiles per DMA chunk (5*128 rows = 1MB bf16 per strip chunk)
NCH = NT // CH       # 10 chunks
F32 = mybir.dt.float32
BF16 = mybir.dt.bfloat16
H1, H2 = 512, 800    # psum-bank split of the 800-wide strip

_CACHE = {}


def _build_program():
    if "nc" in _CACHE:
        return _CACHE["nc"]
    nc = bacc.Bacc("TRN2", target_bir_lowering=False, debug=False, num_devices=M)

    gbT = nc.dram_tensor("gbT", [DB, N], F32, kind="ExternalInput")
    hbT = nc.dram_tensor("hbT", [DB, S], F32, kind="ExternalInput")
    gsT = nc.dram_tensor("gsT", [DS, N], F32, kind="ExternalInput")
    hsT = nc.dram_tensor("hsT", [DS, S], F32, kind="ExternalInput")
    u_cn = nc.dram_tensor("u_cn", [C, S], F32, kind="ExternalInput")
    skcT = nc.dram_tensor("skcT", [C, C], F32, kind="ExternalInput")
    bkcT = nc.dram_tensor("bkcT", [C, C], F32, kind="ExternalInput")
    eye = nc.dram_tensor("eye", [C, C], F32, kind="ExternalInput")
    q_out = nc.dram_tensor("q_out", [C, S], F32, kind="ExternalOutput")

    EXP = mybir.ActivationFunctionType.Exp

    with tile.TileContext(nc) as tc:
        with (
            tc.tile_pool(name="const", bufs=1) as constp,
            tc.tile_pool(name="kwrite", bufs=2) as kwp,
            tc.tile_pool(name="kread", bufs=3) as krp,
            tc.tile_pool(name="smallf", bufs=2) as smallp,
            tc.tile_pool(name="smtile", bufs=4) as smtp,
            tc.tile_pool(name="smfull", bufs=2) as smfp,
            tc.tile_pool(name="qpool", bufs=2) as qp,
            tc.tile_pool(name="dram", bufs=1, space="DRAM") as dramp,
            tc.tile_pool(name="dram_cc", bufs=2, space="DRAM") as dramcc,
        ):
            # ---- resident constants ----
            gb_sb = constp.tile([DB, N], F32, tag="gb")
            nc.sync.dma_start(gb_sb[:], gbT[:, :])
            hb_sb = constp.tile([DB, S], F32, tag="hb")
            nc.sync.dma_start(hb_sb[:], hbT[:, :])
            gs_sb = constp.tile([DS, N], F32, tag="gs")
            nc.sync.dma_start(gs_sb[:], gsT[:, :])
            hs_sb = constp.tile([DS, S], F32, tag="hs")
            nc.sync.dma_start(hs_sb[:], hsT[:, :])
            u_sb = constp.tile([C, S], F32, tag="u")
            nc.sync.dma_start(u_sb[:], u_cn[:, :])
            skc_sb = constp.tile([C, C], F32, tag="skc")
            nc.sync.dma_start(skc_sb[:], skcT[:, :])
            bkc_sb = constp.tile([C, C], F32, tag="bkc")
            nc.sync.dma_start(bkc_sb[:], bkcT[:, :])
            eye_sb = constp.tile([C, C], F32, tag="eye")
            nc.sync.dma_start(eye_sb[:], eye[:, :])
            ones_sb = constp.tile([128, 1], BF16, tag="ones")
            nc.vector.memset(ones_sb[:], 1.0)
            ones_row = constp.tile([1, C], F32, tag="ones_row")
            nc.vector.memset(ones_row[:], 1.0)
            # reciprocal norms, broadcast over the C partitions
            rb_sb = constp.tile([C, S], F32, tag="rb")
            rs_sb = constp.tile([C, S], F32, tag="rs")

            kb_store = dramp.tile([N, S], BF16, tag="kb")
            ks_store = dramp.tile([N, S], BF16, tag="ks")

            # ---- phase 0: materialize K strips (bf16) + norms ----
            with (
                tc.tile_pool(name="psum_ip", bufs=2, space="PSUM") as pip,
                tc.tile_pool(name="psum_norm", bufs=1, space="PSUM") as pnorm,
            ):
              for g_sb, h_sb, store, r_bcast in (
                (gb_sb, hb_sb, kb_store, rb_sb),
                (gs_sb, hs_sb, ks_store, rs_sb),
              ):
                norm_ps = pnorm.tile([1, S], F32, tag="norm")
                for wc in range(NCH):
                    kw = kwp.tile([128, CH, S], BF16, tag="kw")
                    for k in range(CH):
                        t = wc * CH + k
                        ip = pip.tile([128, S], F32, tag="ip")
                        nc.tensor.matmul(
                            ip[:, 0:H1], lhsT=g_sb[:, t * 128:(t + 1) * 128],
                            rhs=h_sb[:, 0:H1], start=True, stop=True)
                        nc.tensor.matmul(
                            ip[:, H1:H2], lhsT=g_sb[:, t * 128:(t + 1) * 128],
                            rhs=h_sb[:, H1:H2], start=True, stop=True)
                        nc.scalar.activation(kw[:, k, :], ip[:, :], EXP)
                        nc.tensor.matmul(
                            norm_ps[0:1, 0:H1], lhsT=ones_sb[:],
                            rhs=kw[:, k, 0:H1],
                            start=(t == 0), stop=(t == NT - 1))
                        nc.tensor.matmul(
                            norm_ps[0:1, H1:H2], lhsT=ones_sb[:],
                            rhs=kw[:, k, H1:H2],
                            start=(t == 0), stop=(t == NT - 1))
                    nc.sync.dma_start(
                        store[wc * CH * 128:(wc + 1) * CH * 128, :]
                        .rearrange("(k p) i -> p k i", p=128),
                        kw[:],
                    )
                r1 = smallp.tile([1, S], F32, tag="r1")
                nc.vector.reciprocal(r1[:], norm_ps[0:1, :])
                # broadcast the reciprocal row across the C partitions via a
                # K=1 matmul with an all-ones column
                bc_ps = pip.tile([C, S], F32, tag="ip")
                nc.tensor.matmul(bc_ps[:, 0:H1], lhsT=ones_row[:],
                                 rhs=r1[0:1, 0:H1], start=True, stop=True)
                nc.tensor.matmul(bc_ps[:, H1:H2], lhsT=ones_row[:],
                                 rhs=r1[0:1, H1:H2], start=True, stop=True)
                nc.vector.tensor_copy(r_bcast[:], bc_ps[:, :])

            # ---- phase 1: mean-field iterations ----
            with (
                tc.tile_pool(name="psum_acc", bufs=1, space="PSUM") as pacc,
                tc.tile_pool(name="psum_tq", bufs=2, space="PSUM") as ptq,
                tc.tile_pool(name="psum_pw", bufs=1, space="PSUM") as ppw,
            ):
              q_cur = u_sb
              for it in range(NIT):
                # softmax over classes for this strip, in (pixel, class)
                # layout; 8 tiles of 100 pixels so the bounce is one DMA
                sm_in = dramcc.tile([S, C], BF16, tag="sm_in")
                smcat = smtp.tile([100, M, C], BF16, tag="smcat")
                for s8 in range(M):
                    off = s8 * 100
                    tq = ptq.tile([128, C], F32, tag="tq")
                    nc.tensor.transpose(
                        tq[0:100, :], q_cur[:, off:off + 100], eye_sb[:])
                    e_sb = smtp.tile([128, C], F32, tag="esb")
                    ssum = smtp.tile([128, 1], F32, tag="ssum")
                    nc.scalar.activation(
                        e_sb[0:100, :], tq[0:100, :], EXP,
                        accum_out=ssum[0:100, 0:1])
                    rsum = smtp.tile([128, 1], F32, tag="rsum")
                    nc.vector.reciprocal(rsum[0:100, 0:1], ssum[0:100, 0:1])
                    nc.vector.tensor_scalar_mul(
                        smcat[0:100, s8, :], e_sb[0:100, :], rsum[0:100, 0:1])
                nc.sync.dma_start(
                    sm_in[:, :].rearrange("(s p) c -> p s c", p=100), smcat[:])

                sm_all = dramcc.tile([N, C], BF16, tag="sm_all")
                nc.gpsimd.collective_compute(
                    "AllGather",
                    mybir.AluOpType.bypass,
                    replica_groups=[list(range(M))],
                    ins=[sm_in[:, :].opt()],
                    outs=[sm_all[:, :].opt()],
                )
                smf = smfp.tile([128, NT, C], BF16, tag="smf")
                nc.sync.dma_start(
                    smf[:], sm_all[:, :].rearrange("(t p) c -> p t c", p=128))
                # filter both kernels against the gathered softmax
                psb = pacc.tile([C, S], F32, tag="psb")
                pss = pacc.tile([C, S], F32, tag="pss")
                for chunk in range(NCH):
                    base = chunk * CH * 128
                    kbt = krp.tile([128, CH, S], BF16, tag="kbt")
                    nc.sync.dma_start(
                        kbt[:],
                        kb_store[base:base + CH * 128, :]
                        .rearrange("(k p) i -> p k i", p=128))
                    kst = krp.tile([128, CH, S], BF16, tag="kst")
                    nc.sync.dma_start(
                        kst[:],
                        ks_store[base:base + CH * 128, :]
                        .rearrange("(k p) i -> p k i", p=128))
                    for k in range(CH):
                        t = chunk * CH + k
                        lhs = smf[:, t, :]
                        st, sp = (t == 0), (t == NT - 1)
                        nc.tensor.matmul(psb[:, 0:H1], lhsT=lhs,
                                         rhs=kbt[:, k, 0:H1], start=st, stop=sp)
                        nc.tensor.matmul(psb[:, H1:H2], lhsT=lhs,
                                         rhs=kbt[:, k, H1:H2], start=st, stop=sp)
                        nc.tensor.matmul(pss[:, 0:H1], lhsT=lhs,
                                         rhs=kst[:, k, 0:H1], start=st, stop=sp)
                        nc.tensor.matmul(pss[:, H1:H2], lhsT=lhs,
                                         rhs=kst[:, k, H1:H2], start=st, stop=sp)

                # normalize, apply fused 21x21 chain, update q
                bn = smallp.tile([C, S], F32, tag="bn")
                nc.vector.tensor_mul(bn[:], psb[:, :], rb_sb[:])
                sn = smallp.tile([C, S], F32, tag="sn")
                nc.vector.tensor_mul(sn[:], pss[:, :], rs_sb[:])
                pw = ppw.tile([C, S], F32, tag="pw")
                nc.tensor.matmul(pw[:, 0:H1], lhsT=skc_sb[:], rhs=sn[:, 0:H1],
                                 start=True, stop=False)
                nc.tensor.matmul(pw[:, 0:H1], lhsT=bkc_sb[:], rhs=bn[:, 0:H1],
                                 start=False, stop=True)
                nc.tensor.matmul(pw[:, H1:H2], lhsT=skc_sb[:], rhs=sn[:, H1:H2],
                                 start=True, stop=False)
                nc.tensor.matmul(pw[:, H1:H2], lhsT=bkc_sb[:], rhs=bn[:, H1:H2],
                                 start=False, stop=True)
                qn = qp.tile([C, S], F32, tag="q")
                nc.vector.tensor_sub(qn[:], u_sb[:], pw[:, :])
                q_cur = qn

              nc.sync.dma_start(q_out[:, :], q_cur[:])

    nc.compile()
    _CACHE["nc"] = nc
    return nc


def _host_prep(unaries, rgb, spatial_kernel, bilateral_kernel,
               compatibility_matrix):
    """Per-core input maps.  Only O(N*d) / O(C^2) work happens here."""
    unaries = np.ascontiguousarray(unaries, dtype=np.float32)
    rgb = np.ascontiguousarray(rgb, dtype=np.float32)
    sk = np.asarray(spatial_kernel, dtype=np.float32)
    bk = np.asarray(bilateral_kernel, dtype=np.float32)
    cm = np.asarray(compatibility_matrix, dtype=np.float32)

    ys, xs = np.meshgrid(np.arange(H, dtype=np.float32),
                         np.arange(W, dtype=np.float32), indexing="ij")
    pos = np.stack([xs.ravel(), ys.ravel()], axis=-1)      # (N, 2)
    img = rgb[0].reshape(N, 3)
    fb = np.concatenate([pos / THETA_ALPHA, img / THETA_BETA], axis=-1)
    fs = pos / THETA_GAMMA

    def aug(f):
        sq = (f * f).sum(-1, dtype=np.float32)
        ones = np.ones((f.shape[0], 1), np.float32)
        g = np.concatenate([f, ones, (-0.5 * sq)[:, None]], -1)
        h = np.concatenate([f, (-0.5 * sq)[:, None], ones], -1)
        return (np.ascontiguousarray(g.T, np.float32),
                np.ascontiguousarray(h.T, np.float32))

    gbT, hbT = aug(fb)           # (7, N)
    gsT, hsT = aug(fs)           # (4, N)

    u_cn = np.ascontiguousarray(unaries[0].reshape(N, C).T, np.float32)
    skcT = np.ascontiguousarray((cm @ sk).T, np.float32)
    bkcT = np.ascontiguousarray((cm @ bk).T, np.float32)
    eye = np.eye(C, dtype=np.float32)

    in_maps = []
    for d in range(M):
        sl = slice(d * S, (d + 1) * S)
        in_maps.append({
            "gbT": gbT,
            "hbT": np.ascontiguousarray(hbT[:, sl]),
            "gsT": gsT,
            "hsT": np.ascontiguousarray(hsT[:, sl]),
            "u_cn": np.ascontiguousarray(u_cn[:, sl]),
            "skcT": skcT,
            "bkcT": bkcT,
            "eye": eye,
        })
    return in_maps


def kernel(unaries, rgb, spatial_kernel, bilateral_kernel,
           compatibility_matrix, _run_kwargs=None):
    _install_birpatch()
    nc = _build_program()
    in_maps = _host_prep(unaries, rgb, spatial_kernel, bilateral_kernel,
                         compatibility_matrix)
    kwargs = dict(_run_kwargs or {})
    res = run_bass_kernel_spmd(nc, in_maps, core_ids=list(range(M)), **kwargs)
    _CACHE["last_results"] = res
    q_full = np.concatenate([res.results[d]["q_out"] for d in range(M)], axis=1)
    return np.ascontiguousarray(q_full.T.reshape(1, H, W, C), dtype=np.float32)



# revision 8
# speedup vs baseline: 1.6786x; 1.6786x over previous
"""CRF-RNN layer (nn_CrfRnnLayer) as an 8-core Trainium2 Bass kernel.

Distribution (sharding_hint): shard the N=H*W pixel dimension across the 8
cores; each core owns a strip of S=N/8=800 output pixels and holds the
(N x S) slices of both Gaussian kernels fully resident in SBUF as fp8.

Phase 0 builds the kernel slices on-device:
  * bilateral: one fp16 matmul per 128-pixel j-tile using a hi/lo split of
    the augmented features g=[f,1,-|f|^2/2], h=[f,-|f|^2/2,1] stacked as
    [g_hi;g_hi;g_lo] . [h_hi;h_lo;h_hi] (21-dim contraction) -> exact to
    ~1e-2 in d^2 at full bf16-class PE speed; Exp on the scalar engine
    writes fp8 tiles straight into SBUF.
  * spatial: exact integer arithmetic in fp16 via
    g=[x'^2,y'^2,1,1,2x',2y'], h=[1,1,x'^2,y'^2,-x',-y'] with per-core
    centered coordinates, Exp(scale=-1/18).
  * norms Sum_j K[j,i] via fp8 DoubleRow matmuls against ones, transposed
    into per-pixel-block scalars, negated reciprocals cached.

Phase 1 runs the 5 mean-field iterations with q kept in (pixel, class)
layout: local softmax -> fp8 AllGather of softmax (padded to 32 cols so
every DMA is contiguous) -> fp8 DoubleRow filter matmuls against the
SBUF-resident kernels (norm folded in afterwards as per-partition scalars)
-> compatibility fold via small matmuls (compat pre-multiplied into the
kernel-weight matrices on the host) -> q update.  The first iteration's
softmax+AllGather is issued before phase 0 so the collective overlaps the
kernel build.
"""

import json

import numpy as np

from concourse import bacc, bass, mybir, tile
from concourse.bass_utils import run_bass_kernel_spmd

H = W = 80
C = 21
CP = 32              # padded class dim (fp8 DoubleRow needs 16B-aligned strides)
N = H * W            # 6400
M = 8                # cores
S = N // M           # 800 pixels per strip
NIT = 5
NS2 = N // 256       # 25 super-tiles of 256 pixels (DoubleRow pairs)
THETA_ALPHA, THETA_BETA, THETA_GAMMA = 160.0, 3.0, 3.0
F32 = mybir.dt.float32
F16 = mybir.dt.float16
FP8 = mybir.dt.float8e4
H1 = 512             # psum-bank split of the 800-wide strip

_CACHE = {}


def _split_bir_multiwaits(bir_json: bytes) -> bytes:
    """Split >1-sync-wait instructions into single-wait chains.

    The staged walrus build allows only one embedded sync-wait per
    instruction; prepend pure-wait EventSemaphores (same engine, same
    block) for all but the last wait.  Tile completion semaphores only
    count up within the kernel epoch, so waiting sequentially is
    equivalent to the simultaneous multi-wait.
    """
    d = json.loads(bir_json)
    for fn in d.get("functions", []):
        for blk in fn.get("blocks", []):
            out = []
            for inst in blk.get("instructions", []):
                si = inst.get("sync_info") or {}
                waits = si.get("on_wait") or []
                if len(waits) > 1:
                    for j, w in enumerate(waits[:-1]):
                        out.append({
                            "debug": inst.get("debug", 0),
                            "engine": inst["engine"],
                            "ins": [],
                            "name": f"{inst['name']}-sw{j}",
                            "opcode": "EventSemaphore",
                            "outs": [],
                            "sync_info": {"on_update": [], "on_wait": [w]},
                        })
                    si["on_wait"] = [waits[-1]]
                out.append(inst)
            blk["instructions"] = out
    return json.dumps(d).encode()


def _install_birpatch():
    if _CACHE.get("birpatch"):
        return
    from concourse import bass2jax
    orig = bass2jax.compile_bir_kernel

    def patched(bir_json, tmpdir, neff_name="file.neff"):
        return orig(_split_bir_multiwaits(bir_json), tmpdir, neff_name)

    bass2jax.compile_bir_kernel = patched
    _CACHE["birpatch"] = True


def _build_program(nit=NIT):
    key = ("nc", nit)
    if key in _CACHE:
        return _CACHE[key]
    nc = bacc.Bacc("TRN2", target_bir_lowering=False, debug=False, num_devices=M)

    gbT = nc.dram_tensor("gbT", [C, N], F16, kind="ExternalInput")
    hbT = nc.dram_tensor("hbT", [C, S], F16, kind="ExternalInput")
    gsT = nc.dram_tensor("gsT", [6, N], F16, kind="ExternalInput")
    hsT = nc.dram_tensor("hsT", [6, S], F16, kind="ExternalInput")
    u_px = nc.dram_tensor("u_px", [100, 8, C], F32, kind="ExternalInput")
    skcT = nc.dram_tensor("skcT", [C, C], F16, kind="ExternalInput")
    bkcT = nc.dram_tensor("bkcT", [C, C], F16, kind="ExternalInput")
    q_out = nc.dram_tensor("q_out", [100, 8, C], F32, kind="ExternalOutput")

    EXP = mybir.ActivationFunctionType.Exp
    COPY = mybir.ActivationFunctionType.Copy
    DR = mybir.MatmulPerfMode.DoubleRow

    with tile.TileContext(nc) as tc:
        with (
            tc.tile_pool(name="const", bufs=1) as constp,
            tc.tile_pool(name="smtile", bufs=2) as smtp,
            tc.tile_pool(name="smfull", bufs=2) as smfp,
            tc.tile_pool(name="fcopy", bufs=2) as fcp,
            tc.tile_pool(name="t12", bufs=2) as t12p,
            tc.tile_pool(name="qpool", bufs=2) as qp,
            tc.tile_pool(name="dram_cc", bufs=2, space="DRAM") as dramcc,
        ):
            # ---- resident constants ----
            gb_sb = constp.tile([C, N], F16, tag="gb")
            nc.sync.dma_start(gb_sb[:], gbT[:, :])
            hb_sb = constp.tile([C, S], F16, tag="hb")
            nc.sync.dma_start(hb_sb[:], hbT[:, :])
            gs_sb = constp.tile([6, N], F16, tag="gs")
            nc.sync.dma_start(gs_sb[:], gsT[:, :])
            hs_sb = constp.tile([6, S], F16, tag="hs")
            nc.sync.dma_start(hs_sb[:], hsT[:, :])
            u_sb = constp.tile([100, 8, C], F32, tag="u")
            nc.sync.dma_start(u_sb[:], u_px[:, :, :])
            skc_sb = constp.tile([C, C], F16, tag="skc")
            nc.sync.dma_start(skc_sb[:], skcT[:, :])
            bkc_sb = constp.tile([C, C], F16, tag="bkc")
            nc.sync.dma_start(bkc_sb[:], bkcT[:, :])
            ones2 = constp.tile([128, 2, 16], FP8, tag="ones2")
            nc.vector.memset(ones2[:], 1.0)
            one1 = constp.tile([1, 1], F32, tag="one1")
            nc.vector.memset(one1[:], 1.0)

            kb_sb = constp.tile([128, NS2, 2, S], FP8, tag="kb")
            ks_sb = constp.tile([128, NS2, 2, S], FP8, tag="ks")
            nb_row = constp.tile([1, S], F32, tag="nbrow")
            ns_row = constp.tile([1, S], F32, tag="nsrow")
            rbn_sb = constp.tile([100, 8], F32, tag="rbn")
            rsn_sb = constp.tile([100, 8], F32, tag="rsn")

            def softmax_and_gather(q_tile):
                smcat = smtp.tile([100, 8, CP], FP8, tag="smcat")
                nc.vector.memset(smcat[:, :, C:CP], 0.0)
                for s8 in range(8):
                    esb = smtp.tile([100, C], F32, tag="esb")
                    ssum = smtp.tile([100, 1], F32, tag="ssum")
                    nc.scalar.activation(esb[:], q_tile[:, s8, :], EXP,
                                         accum_out=ssum[:, 0:1])
                    rsum = smtp.tile([100, 1], F32, tag="rsum")
                    nc.vector.reciprocal(rsum[:, 0:1], ssum[:, 0:1])
                    nc.vector.tensor_scalar_mul(
                        smcat[:, s8, 0:C], esb[:], rsum[:, 0:1])
                sm_in = dramcc.tile([S, CP], FP8, tag="sm_in")
                nc.sync.dma_start(
                    sm_in[:, :].rearrange("(s p) c -> p s c", p=100), smcat[:])
                sm_all = dramcc.tile([N, CP], FP8, tag="sm_all")
                nc.gpsimd.collective_compute(
                    "AllGather",
                    mybir.AluOpType.bypass,
                    replica_groups=[list(range(M))],
                    ins=[sm_in[:, :].opt()],
                    outs=[sm_all[:, :].opt()],
                )
                return sm_all

            # iteration-1 softmax+gather first: overlaps phase 0
            sm_all = softmax_and_gather(u_sb)

            # ---- phase 0: materialize fp8 kernel slices in SBUF + norms ----
            with (
                tc.tile_pool(name="psum_ip", bufs=2, space="PSUM") as pip,
                tc.tile_pool(name="psum_norm", bufs=1, space="PSUM") as pnorm,
                tc.tile_pool(name="psum_tr", bufs=1, space="PSUM") as ptr,
            ):
                for g_sb, h_sb, k_sb, scale, nrow in (
                    (gb_sb, hb_sb, kb_sb, 1.0, nb_row),
                    (gs_sb, hs_sb, ks_sb, -1.0 / 18.0, ns_row),
                ):
                    norm_ps = pnorm.tile([1, S], F32, tag="norm")
                    for st in range(NS2):
                        for t2 in range(2):
                            T = st * 2 + t2
                            ip = pip.tile([128, S], F32, tag="ip")
                            nc.tensor.matmul(
                                ip[:, 0:H1],
                                lhsT=g_sb[:, T * 128:(T + 1) * 128],
                                rhs=h_sb[:, 0:H1], start=True, stop=True)
                            nc.tensor.matmul(
                                ip[:, H1:S],
                                lhsT=g_sb[:, T * 128:(T + 1) * 128],
                                rhs=h_sb[:, H1:S], start=True, stop=True)
                            nc.scalar.activation(
                                k_sb[:, st, t2, :], ip[:, :], EXP, scale=scale)
                        # norm accumulation rides in the PE gaps behind Exp
                        nc.tensor.matmul(
                            norm_ps[0:1, 0:H1], lhsT=ones2[:, :, 0:1],
                            rhs=k_sb[:, st, :, 0:H1],
                            start=(st == 0), stop=(st == NS2 - 1),
                            perf_mode=DR)
                        nc.tensor.matmul(
                            norm_ps[0:1, H1:S], lhsT=ones2[:, :, 0:1],
                            rhs=k_sb[:, st, :, H1:S],
                            start=(st == 0), stop=(st == NS2 - 1),
                            perf_mode=DR)
                    # negate into SBUF row
                    nc.scalar.activation(nrow[:], norm_ps[0:1, :], COPY,
                                         scale=-1.0)
                # transpose the negated norm rows into per-block scalars,
                # then reciprocal -> -1/norm
                for nrow, rout in ((nb_row, rbn_sb), (ns_row, rsn_sb)):
                    trp = ptr.tile([100, 8], F32, tag="tr")
                    for s8 in range(8):
                        nc.tensor.transpose(
                            trp[:, s8:s8 + 1],
                            nrow[0:1, s8 * 100:(s8 + 1) * 100], one1[:])
                    nc.vector.reciprocal(rout[:], trp[:, :])

            # ---- phase 1: mean-field iterations ----
            with (
                tc.tile_pool(name="psum_acc", bufs=1, space="PSUM") as pacc,
                tc.tile_pool(name="psum_pw", bufs=4, space="PSUM") as ppw,
            ):
                q_cur = u_sb
                for it in range(nit):
                    smf = smfp.tile([128, NS2, 2, CP], FP8, tag="smf")
                    nc.sync.dma_start(
                        smf[:],
                        sm_all[:, :].rearrange("(s t p) c -> p s t c",
                                               p=128, t=2))
                    psb = pacc.tile([C, S], F32, tag="psb")
                    pss = pacc.tile([C, S], F32, tag="pss")
                    for st in range(NS2):
                        lhs = smf[:, st, :, 0:C]
                        st_f, sp_f = (st == 0), (st == NS2 - 1)
                        nc.tensor.matmul(psb[:, 0:H1], lhsT=lhs,
                                         rhs=kb_sb[:, st, :, 0:H1],
                                         start=st_f, stop=sp_f, perf_mode=DR)
                        nc.tensor.matmul(psb[:, H1:S], lhsT=lhs,
                                         rhs=kb_sb[:, st, :, H1:S],
                                         start=st_f, stop=sp_f, perf_mode=DR)
                        nc.tensor.matmul(pss[:, 0:H1], lhsT=lhs,
                                         rhs=ks_sb[:, st, :, 0:H1],
                                         start=st_f, stop=sp_f, perf_mode=DR)
                        nc.tensor.matmul(pss[:, H1:S], lhsT=lhs,
                                         rhs=ks_sb[:, st, :, H1:S],
                                         start=st_f, stop=sp_f, perf_mode=DR)
                    # copy raw filter outputs to SBUF (fp16) as pairwise lhsT;
                    # split column-halves across DVE and GpSimd to halve latency
                    fb_sb = fcp.tile([C, S], F16, tag="fb")
                    nc.vector.tensor_copy(fb_sb[:, 0:H1], psb[:, 0:H1])
                    nc.scalar.copy(fb_sb[:, H1:S], psb[:, H1:S])
                    fs_sb = fcp.tile([C, S], F16, tag="fs")
                    nc.vector.tensor_copy(fs_sb[:, 0:H1], pss[:, 0:H1])
                    nc.scalar.copy(fs_sb[:, H1:S], pss[:, H1:S])
                    # pairwise fold + normalization + q update, px-block layout
                    tmp_sb = t12p.tile([100, 8, C], F32, tag="tmp")
                    qn = qp.tile([100, 8, C], F32, tag="qn")
                    MUL = mybir.AluOpType.mult
                    ADD = mybir.AluOpType.add
                    for s8 in range(8):
                        sl = slice(s8 * 100, (s8 + 1) * 100)
                        pwS = ppw.tile([100, C], F32, tag="pw")
                        nc.tensor.matmul(pwS[:], lhsT=fs_sb[:, sl],
                                         rhs=skc_sb[:], start=True, stop=True)
                        pwB = ppw.tile([100, C], F32, tag="pw")
                        nc.tensor.matmul(pwB[:], lhsT=fb_sb[:, sl],
                                         rhs=bkc_sb[:], start=True, stop=True)
                        # tmp = pwB * (-1/norm_b) + u ; qn = pwS * (-1/norm_s) + tmp
                        nc.vector.scalar_tensor_tensor(
                            tmp_sb[:, s8, :], pwB[:], rbn_sb[:, s8:s8 + 1],
                            u_sb[:, s8, :], op0=MUL, op1=ADD)
                        nc.vector.scalar_tensor_tensor(
                            qn[:, s8, :], pwS[:], rsn_sb[:, s8:s8 + 1],
                            tmp_sb[:, s8, :], op0=MUL, op1=ADD)
                    q_cur = qn
                    if it < nit - 1:
                        sm_all = softmax_and_gather(qn)
                nc.sync.dma_start(q_out[:, :, :], q_cur[:])

    nc.compile()
    _CACHE[key] = nc
    return nc


def _host_prep(unaries, rgb, spatial_kernel, bilateral_kernel,
               compatibility_matrix):
    unaries = np.ascontiguousarray(unaries, dtype=np.float32)
    rgb = np.ascontiguousarray(rgb, dtype=np.float32)
    sk = np.asarray(spatial_kernel, dtype=np.float32)
    bk = np.asarray(bilateral_kernel, dtype=np.float32)
    cm = np.asarray(compatibility_matrix, dtype=np.float32)

    ys, xs = np.meshgrid(np.arange(H, dtype=np.float64),
                         np.arange(W, dtype=np.float64), indexing="ij")
    xs, ys = xs.ravel(), ys.ravel()                     # (N,) pixel coords
    img = rgb[0].reshape(N, 3).astype(np.float64)

    # bilateral: hi/lo fp16 split of augmented features
    fb = np.concatenate([xs[:, None] / THETA_ALPHA, ys[:, None] / THETA_ALPHA,
                         img / THETA_BETA], axis=1)     # (N, 5) f64
    sq = 0.5 * (fb * fb).sum(axis=1)
    onesN = np.ones((N, 1))
    g7 = np.concatenate([fb, onesN, -sq[:, None]], axis=1)   # (N, 7)
    h7 = np.concatenate([fb, -sq[:, None], onesN], axis=1)   # (N, 7)

    def split(a):
        hi = a.astype(np.float16)
        lo = (a - hi.astype(np.float64)).astype(np.float16)
        return hi, lo

    g_hi, g_lo = split(g7)
    h_hi, h_lo = split(h7)
    gb21 = np.concatenate([g_hi, g_hi, g_lo], axis=1)   # (N, 21)
    hb21 = np.concatenate([h_hi, h_lo, h_hi], axis=1)   # (N, 21)
    gbT = np.ascontiguousarray(gb21.T)                  # (21, N) f16

    # spatial: exact integer features, per-core centered y
    xi = xs - 40.0                                      # |x'| <= 40
    u_cn = unaries[0].reshape(N, C)
    skcT = np.ascontiguousarray((cm @ sk).T.astype(np.float16))
    bkcT = np.ascontiguousarray((cm @ bk).T.astype(np.float16))

    in_maps = []
    for d in range(M):
        sl = slice(d * S, (d + 1) * S)
        yi = ys - (10.0 * d + 5.0)                      # per-core centered
        gs6 = np.stack([xi * xi, yi * yi, np.ones(N), np.ones(N),
                        2.0 * xi, 2.0 * yi], axis=0).astype(np.float16)
        hs6 = np.stack([np.ones(S), np.ones(S),
                        (xi * xi)[sl], (yi * yi)[sl],
                        -xi[sl], -yi[sl]], axis=0).astype(np.float16)
        u_strip = u_cn[sl].reshape(8, 100, C).transpose(1, 0, 2)
        in_maps.append({
            "gbT": gbT,
            "hbT": np.ascontiguousarray(hb21[sl].T),
            "gsT": np.ascontiguousarray(gs6),
            "hsT": np.ascontiguousarray(hs6),
            "u_px": np.ascontiguousarray(u_strip),
            "skcT": skcT,
            "bkcT": bkcT,
        })
    return in_maps


def kernel(unaries, rgb, spatial_kernel, bilateral_kernel,
           compatibility_matrix, _run_kwargs=None):
    _install_birpatch()
    nc = _build_program()
    in_maps = _host_prep(unaries, rgb, spatial_kernel, bilateral_kernel,
                         compatibility_matrix)
    kwargs = dict(_run_kwargs or {})
    res = run_bass_kernel_spmd(nc, in_maps, core_ids=list(range(M)), **kwargs)
    _CACHE["last_results"] = res
    strips = [res.results[d]["q_out"].transpose(1, 0, 2).reshape(S, C)
              for d in range(M)]
    q_full = np.concatenate(strips, axis=0)             # (N, C)
    return np.ascontiguousarray(q_full.reshape(1, H, W, C), dtype=np.float32)


# revision 20
# speedup vs baseline: 1.8753x; 1.1171x over previous
"""CRF-RNN layer (nn_CrfRnnLayer) as an 8-core Trainium2 Bass kernel.

Distribution (sharding_hint): shard the N=H*W pixel dimension across the 8
cores; each core owns a strip of S=N/8=800 output pixels and holds the
(N x S) slices of both Gaussian kernels fully resident in SBUF as fp8.

Phase 0 builds the kernel slices on-device:
  * bilateral: one fp16 matmul per 128-pixel j-tile using a hi/lo split of
    the augmented features g=[f,1,-|f|^2/2], h=[f,-|f|^2/2,1] stacked as
    [g_hi;g_hi;g_lo] . [h_hi;h_lo;h_hi] (21-dim contraction) -> exact to
    ~1e-2 in d^2 at full bf16-class PE speed; Exp on the scalar engine
    writes fp8 tiles straight into SBUF.
  * spatial: exact integer arithmetic in fp16 via
    g=[x'^2,y'^2,1,1,2x',2y'], h=[1,1,x'^2,y'^2,-x',-y'] with per-core
    centered coordinates, Exp(scale=-1/18).
  * norms Sum_j K[j,i] via fp8 DoubleRow matmuls against ones, transposed
    into per-pixel-block scalars, negated reciprocals cached.

Phase 1 runs the 5 mean-field iterations with q kept in (pixel, class)
layout: local softmax -> fp8 AllGather of softmax (padded to 32 cols so
every DMA is contiguous) -> fp8 DoubleRow filter matmuls against the
SBUF-resident kernels (norm folded in afterwards as per-partition scalars)
-> compatibility fold via small matmuls (compat pre-multiplied into the
kernel-weight matrices on the host) -> q update.  The first iteration's
softmax+AllGather is issued before phase 0 so the collective overlaps the
kernel build.
"""

import json

import numpy as np

from concourse import bacc, bass, mybir, tile
from concourse.bass_utils import run_bass_kernel_spmd

H = W = 80
C = 21
CP = 32              # padded class dim (fp8 DoubleRow needs 16B-aligned strides)
N = H * W            # 6400
M = 8                # cores
S = N // M           # 800 pixels per strip
NIT = 5
NS2 = N // 256       # 25 super-tiles of 256 pixels (DoubleRow pairs)
THETA_ALPHA, THETA_BETA, THETA_GAMMA = 160.0, 3.0, 3.0
F32 = mybir.dt.float32
F16 = mybir.dt.float16
FP8 = mybir.dt.float8e4
H1 = 512             # psum-bank split of the 800-wide strip

_CACHE = {}


def _split_bir_multiwaits(bir_json: bytes) -> bytes:
    """Split >1-sync-wait instructions into single-wait chains.

    The staged walrus build allows only one embedded sync-wait per
    instruction; prepend pure-wait EventSemaphores (same engine, same
    block) for all but the last wait.  Tile completion semaphores only
    count up within the kernel epoch, so waiting sequentially is
    equivalent to the simultaneous multi-wait.
    """
    d = json.loads(bir_json)
    for fn in d.get("functions", []):
        for blk in fn.get("blocks", []):
            out = []
            for inst in blk.get("instructions", []):
                si = inst.get("sync_info") or {}
                waits = si.get("on_wait") or []
                if len(waits) > 1:
                    for j, w in enumerate(waits[:-1]):
                        out.append({
                            "debug": inst.get("debug", 0),
                            "engine": inst["engine"],
                            "ins": [],
                            "name": f"{inst['name']}-sw{j}",
                            "opcode": "EventSemaphore",
                            "outs": [],
                            "sync_info": {"on_update": [], "on_wait": [w]},
                        })
                    si["on_wait"] = [waits[-1]]
                out.append(inst)
            blk["instructions"] = out
    return json.dumps(d).encode()


def _install_birpatch():
    if _CACHE.get("birpatch"):
        return
    from concourse import bass2jax
    orig = bass2jax.compile_bir_kernel

    def patched(bir_json, tmpdir, neff_name="file.neff"):
        return orig(_split_bir_multiwaits(bir_json), tmpdir, neff_name)

    bass2jax.compile_bir_kernel = patched
    _CACHE["birpatch"] = True


def _build_program(nit=NIT):
    key = ("nc", nit)
    if key in _CACHE:
        return _CACHE[key]
    nc = bacc.Bacc("TRN2", target_bir_lowering=False, debug=False, num_devices=M)

    gbT = nc.dram_tensor("gbT", [C, N], F16, kind="ExternalInput")
    hbT = nc.dram_tensor("hbT", [C, S], F16, kind="ExternalInput")
    gsT = nc.dram_tensor("gsT", [6, N], F16, kind="ExternalInput")
    hsT = nc.dram_tensor("hsT", [6, S], F16, kind="ExternalInput")
    u_px = nc.dram_tensor("u_px", [100, 8, C], F32, kind="ExternalInput")
    skbkT = nc.dram_tensor("skbkT", [53, C], F16, kind="ExternalInput")
    q_out = nc.dram_tensor("q_out", [100, 8, C], F32, kind="ExternalOutput")

    EXP = mybir.ActivationFunctionType.Exp
    COPY = mybir.ActivationFunctionType.Copy
    DR = mybir.MatmulPerfMode.DoubleRow

    with tile.TileContext(nc) as tc:
        with (
            tc.tile_pool(name="const", bufs=1) as constp,
            tc.tile_pool(name="smtile", bufs=2) as smtp,
            tc.tile_pool(name="smfull", bufs=2) as smfp,
            tc.tile_pool(name="fcopy", bufs=2) as fcp,
            tc.tile_pool(name="qpool", bufs=2) as qp,
            tc.tile_pool(name="dram_cc", bufs=2, space="DRAM") as dramcc,
        ):
            # ---- resident constants ----
            gb_sb = constp.tile([C, N], F16, tag="gb")
            nc.sync.dma_start(gb_sb[:], gbT[:, :])
            hb_sb = constp.tile([C, S], F16, tag="hb")
            nc.sync.dma_start(hb_sb[:], hbT[:, :])
            gs_sb = constp.tile([6, N], F16, tag="gs")
            nc.sync.dma_start(gs_sb[:], gsT[:, :])
            hs_sb = constp.tile([6, S], F16, tag="hs")
            nc.sync.dma_start(hs_sb[:], hsT[:, :])
            u_sb = constp.tile([100, 8, C], F32, tag="u")
            nc.sync.dma_start(u_sb[:], u_px[:, :, :])
            skbk_sb = constp.tile([53, C], F16, tag="skbk")
            nc.sync.dma_start(skbk_sb[:], skbkT[:, :])
            ones2 = constp.tile([128, 2, 16], FP8, tag="ones2")
            nc.vector.memset(ones2[:], 1.0)
            onesC = constp.tile([1, C], F16, tag="onesC")
            nc.vector.memset(onesC[:], 1.0)

            kb_sb = constp.tile([128, NS2, 2, S], FP8, tag="kb")
            ks_sb = constp.tile([128, NS2, 2, S], FP8, tag="ks")
            # broadcast -1/norm rows, (C, S) per kernel
            rbc_sb = constp.tile([C, S], F32, tag="rbc")
            rsc_sb = constp.tile([C, S], F32, tag="rsc")

            X = mybir.AxisListType.X
            ADD = mybir.AluOpType.add

            def softmax_and_gather(q_tile):
                smcat = smtp.tile([100, 8, CP], FP8, tag="smcat")
                nc.vector.memset(smcat[:, :, C:CP], 0.0)
                esb = smtp.tile([100, 8, C], F32, tag="esb")
                nc.scalar.activation(esb[:], q_tile[:], EXP)
                ssum = smtp.tile([100, 8], F32, tag="ssum")
                nc.vector.tensor_reduce(ssum[:], esb[:], X, ADD)
                rsum = smtp.tile([100, 8], F32, tag="rsum")
                nc.vector.reciprocal(rsum[:], ssum[:])
                for s8 in range(8):
                    nc.vector.tensor_scalar_mul(
                        smcat[:, s8, 0:C], esb[:, s8, :], rsum[:, s8:s8 + 1])
                sm_in = dramcc.tile([S, CP], FP8, tag="sm_in")
                nc.sync.dma_start(
                    sm_in[:, :].rearrange("(s p) c -> p s c", p=100), smcat[:])
                sm_all = dramcc.tile([N, CP], FP8, tag="sm_all")
                nc.gpsimd.collective_compute(
                    "AllGather",
                    mybir.AluOpType.bypass,
                    replica_groups=[list(range(M))],
                    ins=[sm_in[:, :].opt()],
                    outs=[sm_all[:, :].opt()],
                )
                return sm_all

            # iteration-1 softmax+gather first: overlaps phase 0
            sm_all = softmax_and_gather(u_sb)

            # ---- phase 0: materialize fp8 kernel slices in SBUF + norms ----
            with (
                tc.tile_pool(name="psum_ip", bufs=2, space="PSUM") as pip,
                tc.tile_pool(name="psum_norm", bufs=1, space="PSUM") as pnorm,
                tc.tile_pool(name="psum_bc", bufs=1, space="PSUM") as pbc,
            ):
                for g_sb, h_sb, k_sb, scale, rbcast in (
                    (gb_sb, hb_sb, kb_sb, 1.0, rbc_sb),
                    (gs_sb, hs_sb, ks_sb, -1.0 / 18.0, rsc_sb),
                ):
                    norm_ps = pnorm.tile([1, S], F32, tag="norm")
                    for st in range(NS2):
                        for t2 in range(2):
                            T = st * 2 + t2
                            ip = pip.tile([128, S], F32, tag="ip")
                            nc.tensor.matmul(
                                ip[:, 0:H1],
                                lhsT=g_sb[:, T * 128:(T + 1) * 128],
                                rhs=h_sb[:, 0:H1], start=True, stop=True)
                            nc.tensor.matmul(
                                ip[:, H1:S],
                                lhsT=g_sb[:, T * 128:(T + 1) * 128],
                                rhs=h_sb[:, H1:S], start=True, stop=True)
                            nc.scalar.activation(
                                k_sb[:, st, t2, :], ip[:, :], EXP, scale=scale)
                        # norm accumulation rides in the PE gaps behind Exp
                        nc.tensor.matmul(
                            norm_ps[0:1, 0:H1], lhsT=ones2[:, :, 0:1],
                            rhs=k_sb[:, st, :, 0:H1],
                            start=(st == 0), stop=(st == NS2 - 1),
                            perf_mode=DR)
                        nc.tensor.matmul(
                            norm_ps[0:1, H1:S], lhsT=ones2[:, :, 0:1],
                            rhs=k_sb[:, st, :, H1:S],
                            start=(st == 0), stop=(st == NS2 - 1),
                            perf_mode=DR)
                    # -1/norm broadcast to all C partitions:
                    # reciprocal -> negate (fp16) -> K=1 matmul broadcast
                    nr_sb = smtp.tile([1, S], F32, tag="nr")
                    nc.vector.reciprocal(nr_sb[:], norm_ps[0:1, :])
                    nr16 = smtp.tile([1, S], F16, tag="nr16")
                    nc.scalar.activation(nr16[:], nr_sb[:], COPY, scale=-1.0)
                    bc_ps = pbc.tile([C, S], F32, tag="bc")
                    nc.tensor.matmul(bc_ps[:, 0:H1], lhsT=onesC[:],
                                     rhs=nr16[0:1, 0:H1], start=True, stop=True)
                    nc.tensor.matmul(bc_ps[:, H1:S], lhsT=onesC[:],
                                     rhs=nr16[0:1, H1:S], start=True, stop=True)
                    nc.vector.tensor_copy(rbcast[:], bc_ps[:, :])

            # ---- phase 1: mean-field iterations ----
            with (
                tc.tile_pool(name="psum_acc", bufs=1, space="PSUM") as pacc,
                tc.tile_pool(name="psum_pw", bufs=2, space="PSUM") as ppw,
            ):
                q_cur = u_sb
                for it in range(nit):
                    smf = smfp.tile([128, NS2, 2, CP], FP8, tag="smf")
                    nc.sync.dma_start(
                        smf[:],
                        sm_all[:, :].rearrange("(s t p) c -> p s t c",
                                               p=128, t=2))
                    psb = pacc.tile([C, S], F32, tag="psb")
                    pss = pacc.tile([C, S], F32, tag="pss")
                    for st in range(NS2):
                        lhs = smf[:, st, :, 0:C]
                        st_f, sp_f = (st == 0), (st == NS2 - 1)
                        nc.tensor.matmul(psb[:, 0:H1], lhsT=lhs,
                                         rhs=kb_sb[:, st, :, 0:H1],
                                         start=st_f, stop=sp_f, perf_mode=DR)
                        nc.tensor.matmul(psb[:, H1:S], lhsT=lhs,
                                         rhs=kb_sb[:, st, :, H1:S],
                                         start=st_f, stop=sp_f, perf_mode=DR)
                        nc.tensor.matmul(pss[:, 0:H1], lhsT=lhs,
                                         rhs=ks_sb[:, st, :, 0:H1],
                                         start=st_f, stop=sp_f, perf_mode=DR)
                        nc.tensor.matmul(pss[:, H1:S], lhsT=lhs,
                                         rhs=ks_sb[:, st, :, H1:S],
                                         start=st_f, stop=sp_f, perf_mode=DR)
                    # normalized filter outputs -> stacked fp16 pairwise lhsT:
                    # rows 0:21 spatial*(-1/norm_s), rows 21:42 bilateral*(-1/norm_b)
                    fsb = fcp.tile([53, S], F16, tag="fsb")
                    nc.vector.memset(fsb[0:32, :], 0.0)
                    nc.vector.tensor_mul(fsb[0:C, 0:H1], pss[:, 0:H1],
                                         rsc_sb[:, 0:H1])
                    nc.vector.tensor_mul(fsb[0:C, H1:S], pss[:, H1:S],
                                         rsc_sb[:, H1:S])
                    nc.vector.tensor_mul(fsb[32:53, 0:H1], psb[:, 0:H1],
                                         rbc_sb[:, 0:H1])
                    nc.vector.tensor_mul(fsb[32:53, H1:S], psb[:, H1:S],
                                         rbc_sb[:, H1:S])
                    # pairwise fold: one 42-dim matmul per pixel block
                    pw = ppw.tile([100, 8, C], F32, tag="pw")
                    for s8 in range(8):
                        sl = slice(s8 * 100, (s8 + 1) * 100)
                        nc.tensor.matmul(pw[:, s8, :], lhsT=fsb[:, sl],
                                         rhs=skbk_sb[:], start=True, stop=True)
                    qn = qp.tile([100, 8, C], F32, tag="qn")
                    nc.vector.tensor_add(qn[:], u_sb[:], pw[:, :, :])
                    q_cur = qn
                    if it < nit - 1:
                        sm_all = softmax_and_gather(qn)
                nc.sync.dma_start(q_out[:, :, :], q_cur[:])

    nc.compile()
    _CACHE[key] = nc
    return nc


def _host_prep(unaries, rgb, spatial_kernel, bilateral_kernel,
               compatibility_matrix):
    unaries = np.ascontiguousarray(unaries, dtype=np.float32)
    rgb = np.ascontiguousarray(rgb, dtype=np.float32)
    sk = np.asarray(spatial_kernel, dtype=np.float32)
    bk = np.asarray(bilateral_kernel, dtype=np.float32)
    cm = np.asarray(compatibility_matrix, dtype=np.float32)

    ys, xs = np.meshgrid(np.arange(H, dtype=np.float64),
                         np.arange(W, dtype=np.float64), indexing="ij")
    xs, ys = xs.ravel(), ys.ravel()                     # (N,) pixel coords
    img = rgb[0].reshape(N, 3).astype(np.float64)

    # bilateral: hi/lo fp16 split of augmented features
    fb = np.concatenate([xs[:, None] / THETA_ALPHA, ys[:, None] / THETA_ALPHA,
                         img / THETA_BETA], axis=1)     # (N, 5) f64
    sq = 0.5 * (fb * fb).sum(axis=1)
    onesN = np.ones((N, 1))
    g7 = np.concatenate([fb, onesN, -sq[:, None]], axis=1)   # (N, 7)
    h7 = np.concatenate([fb, -sq[:, None], onesN], axis=1)   # (N, 7)

    def split(a):
        hi = a.astype(np.float16)
        lo = (a - hi.astype(np.float64)).astype(np.float16)
        return hi, lo

    g_hi, g_lo = split(g7)
    h_hi, h_lo = split(h7)
    gb21 = np.concatenate([g_hi, g_hi, g_lo], axis=1)   # (N, 21)
    hb21 = np.concatenate([h_hi, h_lo, h_hi], axis=1)   # (N, 21)
    gbT = np.ascontiguousarray(gb21.T)                  # (21, N) f16

    # spatial: exact integer features, per-core centered y
    xi = xs - 40.0                                      # |x'| <= 40
    u_cn = unaries[0].reshape(N, C)
    skbkT = np.ascontiguousarray(np.concatenate(
        [(cm @ sk).T, np.zeros((11, C), np.float32), (cm @ bk).T],
        axis=0).astype(np.float16))                     # (53, 21), zero gap

    in_maps = []
    for d in range(M):
        sl = slice(d * S, (d + 1) * S)
        yi = ys - (10.0 * d + 5.0)                      # per-core centered
        gs6 = np.stack([xi * xi, yi * yi, np.ones(N), np.ones(N),
                        2.0 * xi, 2.0 * yi], axis=0).astype(np.float16)
        hs6 = np.stack([np.ones(S), np.ones(S),
                        (xi * xi)[sl], (yi * yi)[sl],
                        -xi[sl], -yi[sl]], axis=0).astype(np.float16)
        u_strip = u_cn[sl].reshape(8, 100, C).transpose(1, 0, 2)
        in_maps.append({
            "gbT": gbT,
            "hbT": np.ascontiguousarray(hb21[sl].T),
            "gsT": np.ascontiguousarray(gs6),
            "hsT": np.ascontiguousarray(hs6),
            "u_px": np.ascontiguousarray(u_strip),
            "skbkT": skbkT,
        })
    return in_maps


def kernel(unaries, rgb, spatial_kernel, bilateral_kernel,
           compatibility_matrix, _run_kwargs=None):
    _install_birpatch()
    nc = _build_program()
    in_maps = _host_prep(unaries, rgb, spatial_kernel, bilateral_kernel,
                         compatibility_matrix)
    kwargs = dict(_run_kwargs or {})
    res = run_bass_kernel_spmd(nc, in_maps, core_ids=list(range(M)), **kwargs)
    _CACHE["last_results"] = res
    strips = [res.results[d]["q_out"].transpose(1, 0, 2).reshape(S, C)
              for d in range(M)]
    q_full = np.concatenate(strips, axis=0)             # (N, C)
    return np.ascontiguousarray(q_full.reshape(1, H, W, C), dtype=np.float32)
